# revision 3
# baseline (speedup 1.0000x reference)
"""Trainium2 Bass kernel for nn_AttentionBlock (B=8, T=2048, C=K=V=512).

Quirk: the reference softmax runs over the QUERY axis (axis=1) with a causal
mask. Computing scores transposed, ST[j, i] = k_j . q_i, turns that into a
standard row softmax along the free axis, and the output becomes
out = expST^T @ (V / Z) — which maps directly onto the PE with the
normalization folded into V.

Sharding: data-parallel — one batch element per NeuronCore (8 cores).
Host does layout only (transposes/concat); all FLOPs run on device in
float32r (full PE rate, ~1e-4 matmul relerr).

Per-core phases:
  1. QT = Wq @ xT, KT = Wk @ xT (feature-major), V = x @ Wv^T (token-major,
     bounced to a DRAM scratch to free SBUF).
  2. ST tiles [j:128, i:512] = KT^T-slices @ QT-slices over 4 c-chunks,
     causal mask added via an identity-matmul into PSUM (-1e30), then
     ScalarE exp with fused per-partition row-sum (accum_out) -> expST, Zp.
     Fully-masked tiles are skipped (block-causal sparsity).
  3. Z = sum(Zp), recipZ = 1/Z; Vt = V * recipZ (per token row);
     out[ic] = sum_{jb<=ic} expST[jb]^T-slice @ Vt[jb] accumulated in PSUM.

Walrus workaround: this container's compiler allows at most ONE sync wait
per instruction (two on EventSemaphore). Tile emits more; we split them
onto EventSemaphore carrier instructions post-trace.
"""

import math

import numpy as np

T, C = 2048, 512
KS, VS = 512, 512
B = 8
N_CORES = 8
P = 128
CK = C // P            # 4 contraction chunks
NJ = T // P            # 16 j-chunks (128 tokens each)
NI = T // 512          # 4 i-blocks (512 queries each)
NEG = -1.0e30
SCALE = 1.0 / math.sqrt(float(KS))

# expST packed row offsets: row jb holds (NI - jb//4) tiles of width 512
_ROW_NTILES = [NI - (jb // NI) for jb in range(NJ)]
_ROW_OFF = np.concatenate([[0], np.cumsum([n * 512 for n in _ROW_NTILES])]).astype(int)
EXP_TOT = int(_ROW_OFF[-1])  # 40 * 512 = 20480


def _apply_walrus_fixes():
    import concourse.mybir as mybir
    import concourse.tile as tile
    from concourse.vector_clock import ScopedClock

    def _drain_and_barrier(self, tick_clock, wait_clock):
        drain_inst = self.nc.sync.drain()
        wait_clock.add_sem_waits(
            drain_inst.ins, ScopedClock({None: tick_clock.global_clock})
        )
        si = drain_inst.ins.sync_info
        if si is not None and si.on_wait is not None and len(si.on_wait) > 1:
            extras = list(si.on_wait[1:])
            del si.on_wait[1:]
            for w in extras:
                d2 = self.nc.sync.drain()
                if d2.ins.sync_info is None:
                    d2.ins.sync_info = mybir.SyncInfo(on_wait=[], on_update=[])
                d2.ins.sync_info.on_wait.append(w)

        self.nc.all_engine_barrier()
        assert self.sems is not None
        popped = self.nc._tile_sem_poison_stack.pop()
        assert popped is self._sem_poison
        self.nc.clear_and_free_semaphores(list(self.sems.allocated().values()))
        self.nc.all_engine_barrier()

    tile.TileContext._drain_and_barrier = _drain_and_barrier


_SPLIT_UID = [0]


def _split_excess_waits(nc):
    """Hoist excess sync waits onto EventSemaphore carriers (this walrus build
    allows 1 wait per instruction, 2 on EventSemaphore)."""
    import concourse.mybir as mybir

    for f in nc.m.functions:
        for blk in f.blocks:
            out = []
            changed = False
            for inst in blk.instructions:
                si = inst.sync_info
                cap = 2 if isinstance(inst, mybir.InstEventSemaphore) else 1
                if si is not None and si.on_wait is not None and len(si.on_wait) > cap:
                    extras = list(si.on_wait[: len(si.on_wait) - cap])
                    del si.on_wait[: len(si.on_wait) - cap]
                    for i in range(0, len(extras), 2):
                        _SPLIT_UID[0] += 1
                        ev = mybir.InstEventSemaphore(
                            name=f"wsplit_{_SPLIT_UID[0]}",
                            ins=[],
                            outs=[],
                            sync_info=mybir.SyncInfo(
                                on_wait=extras[i : i + 2], on_update=[]
                            ),
                        )
                        ev.engine = inst.engine
                        nc.register_instruction(ev, overwrite=True)
                        out.append(ev)
                    changed = True
                out.append(inst)
            if changed:
                blk.instructions = out


def _build(with_bias: bool):
    import concourse.bass as bass
    import concourse.mybir as mybir
    import concourse.tile as tile

    F32 = mybir.dt.float32
    F32R = mybir.dt.float32r
    Exp = mybir.ActivationFunctionType.Exp
    X = mybir.AxisListType.X

    nc = bass.Bass(trn_type="TRN2", name="attnblock")
    xT_d = nc.dram_tensor("xT", [C, T], F32, kind="ExternalInput")
    wq_d = nc.dram_tensor("wqT", [C, KS], F32, kind="ExternalInput")
    wk_d = nc.dram_tensor("wkT", [C, KS], F32, kind="ExternalInput")
    wv_d = nc.dram_tensor("wvT", [C, VS], F32, kind="ExternalInput")
    maskE_d = nc.dram_tensor("maskE", [P, 896], F32, kind="ExternalInput")
    ident_d = nc.dram_tensor("ident", [P, P], F32, kind="ExternalInput")
    if with_bias:
        bq_d = nc.dram_tensor("bq", [1, KS], F32, kind="ExternalInput")
        bk_d = nc.dram_tensor("bk", [1, KS], F32, kind="ExternalInput")
        bv_d = nc.dram_tensor("bv", [1, VS], F32, kind="ExternalInput")
        ones_d = nc.dram_tensor("ones", [1, 512], F32, kind="ExternalInput")
    out_d = nc.dram_tensor("out", [T, VS], F32, kind="ExternalOutput")

    xT_r = xT_d[:].rearrange("(o p) t -> p o t", p=P)      # [128, 4, 2048]
    wq_r = wq_d[:].rearrange("(o p) k -> p o k", p=P)      # [128, 4, 512]
    wk_r = wk_d[:].rearrange("(o p) k -> p o k", p=P)
    wv_r = wv_d[:].rearrange("(o p) k -> p o k", p=P)

    with tile.TileContext(nc) as tc:
        with (
            tc.tile_pool(name="big", bufs=1) as big,     # xT -> expST (80KB slot)
            tc.tile_pool(name="qtp", bufs=1) as qtp,     # QT -> V3b
            tc.tile_pool(name="ktp", bufs=1) as ktp,     # KT
            tc.tile_pool(name="wp", bufs=3) as wp,       # wq/wk/wv -> maskE, V3a
            tc.tile_pool(name="vp", bufs=1) as vp,       # V natural (phase 1 only? no: stats)
            tc.tile_pool(name="stage", bufs=4) as stage,
            tc.tile_pool(name="ps", bufs=6, space="PSUM") as ps,
            tc.tile_pool(name="dram", bufs=1, space="DRAM") as dram,
        ):
            # ---------------- load phase ----------------
            xT_sb = big.tile([P, CK, T], F32R, tag="big")
            for tt in range(NI):
                nc.sync.dma_start(
                    xT_sb[:, :, 512 * tt : 512 * (tt + 1)],
                    xT_r[:, :, 512 * tt : 512 * (tt + 1)].bitcast(F32R),
                )
            wq_sb = wp.tile([P, CK, KS], F32R, tag="w8")
            nc.sync.dma_start(wq_sb[:], wq_r.bitcast(F32R))
            wk_sb = wp.tile([P, CK, KS], F32R, tag="w8")
            nc.sync.dma_start(wk_sb[:], wk_r.bitcast(F32R))
            wv_sb = wp.tile([P, CK, VS], F32R, tag="w8")
            nc.sync.dma_start(wv_sb[:], wv_r.bitcast(F32R))

            ident_sb = vp.tile([P, P], F32R, tag="ident")
            nc.sync.dma_start(ident_sb[:], ident_d[:].bitcast(F32R))
            Zp = vp.tile([P, NJ, NI], F32, tag="zp")
            nc.vector.memset(Zp[:], 0.0)
            Z = vp.tile([P, NJ], F32, tag="z")
            recipZ = vp.tile([P, NJ], F32, tag="rz")
            if with_bias:
                bq_sb = vp.tile([1, KS], F32R, tag="bq")
                nc.sync.dma_start(bq_sb[:], bq_d[:].bitcast(F32R))
                bk_sb = vp.tile([1, KS], F32R, tag="bk")
                nc.sync.dma_start(bk_sb[:], bk_d[:].bitcast(F32R))
                bv_sb = vp.tile([1, VS], F32R, tag="bv")
                nc.sync.dma_start(bv_sb[:], bv_d[:].bitcast(F32R))
                ones_sb = vp.tile([1, 512], F32R, tag="ones")
                nc.sync.dma_start(ones_sb[:], ones_d[:].bitcast(F32R))

            V_dram = dram.tile([T, VS], F32)

            QT = qtp.tile([P, CK, T], F32R, tag="qt")    # [k-part, kc, t]
            KT = ktp.tile([P, CK, T], F32R, tag="kt")

            # ---------------- phase 1: QKV ----------------
            for tt in range(NI):
                tsl = slice(512 * tt, 512 * (tt + 1))
                for w_sb, b_sb, dst in (
                    (wq_sb, "bq", QT),
                    (wk_sb, "bk", KT),
                ):
                    for kc in range(CK):
                        pq = ps.tile([P, 512], F32, tag="ps")
                        for cc in range(CK):
                            nc.tensor.matmul(
                                pq[:],
                                w_sb[:, cc, P * kc : P * (kc + 1)],
                                xT_sb[:, cc, tsl],
                                start=(cc == 0),
                                stop=(cc == CK - 1 and not with_bias),
                            )
                        if with_bias:
                            brow = bq_sb if b_sb == "bq" else bk_sb
                            nc.tensor.matmul(
                                pq[:],
                                brow[:, P * kc : P * (kc + 1)],
                                ones_sb[:, :512],
                                start=False,
                                stop=True,
                            )
                        nc.vector.tensor_copy(dst[:, kc, tsl], pq[:])
                # V natural tiles for this t-range
                for tp in range(4):
                    row0 = 512 * tt + P * tp
                    pv = ps.tile([P, 512], F32, tag="ps")
                    for cc in range(CK):
                        nc.tensor.matmul(
                            pv[:],
                            xT_sb[:, cc, row0 : row0 + P],
                            wv_sb[:, cc, :],
                            start=(cc == 0),
                            stop=(cc == CK - 1 and not with_bias),
                        )
                    if with_bias:
                        nc.tensor.matmul(
                            pv[:], ones_sb[:, :P], bv_sb[:], start=False, stop=True
                        )
                    st = stage.tile([P, 512], F32, tag="st")
                    nc.scalar.copy(st[:], pv[:])
                    nc.sync.dma_start(V_dram[row0 : row0 + P, :], st[:])

            # ---------------- phase 2: scores + exp ----------------
            expST = big.tile([P, EXP_TOT], F32R, tag="big")
            maskE_sb = wp.tile([P, 896], F32R, tag="w8")
            nc.sync.dma_start(maskE_sb[:], maskE_d[:].bitcast(F32R))

            for jb in range(NJ):
                ib0 = jb // NI
                jsl = slice(P * jb, P * (jb + 1))
                for ib in range(ib0, NI):
                    pst = ps.tile([P, 512], F32, tag="ps")
                    diag = ib == ib0
                    for cc in range(CK):
                        nc.tensor.matmul(
                            pst[:],
                            KT[:, cc, jsl],
                            QT[:, cc, 512 * ib : 512 * (ib + 1)],
                            start=(cc == 0),
                            stop=(cc == CK - 1 and not diag),
                        )
                    if diag:
                        moff = 384 - P * (jb % NI)
                        nc.tensor.matmul(
                            pst[:],
                            ident_sb[:],
                            maskE_sb[:, moff : moff + 512],
                            start=False,
                            stop=True,
                        )
                    off = int(_ROW_OFF[jb]) + 512 * (ib - ib0)
                    nc.scalar.activation(
                        expST[:, off : off + 512],
                        pst[:],
                        Exp,
                        bias=0.0,
                        scale=SCALE,
                        accum_out=Zp[:, jb, ib : ib + 1],
                    )

            # ---------------- phase 3: normalize V, PV matmul ----------------
            nc.vector.reduce_sum(Z[:, :, None], Zp[:], axis=X)
            nc.vector.reciprocal(recipZ[:], Z[:])

            # V comes back from DRAM: jb 0..7 into two wp slots, 8..15 into qt slot
            v3a0 = wp.tile([P, 4, VS], F32R, tag="w8")
            nc.sync.dma_start(
                v3a0[:].bitcast(F32),
                V_dram[0:512, :].rearrange("(o p) v -> p o v", p=P),
            )
            v3a1 = wp.tile([P, 4, VS], F32R, tag="w8")
            nc.sync.dma_start(
                v3a1[:].bitcast(F32),
                V_dram[512:1024, :].rearrange("(o p) v -> p o v", p=P),
            )
            v3b = qtp.tile([P, 8, VS], F32R, tag="qt")
            nc.sync.dma_start(
                v3b[:].bitcast(F32),
                V_dram[1024:2048, :].rearrange("(o p) v -> p o v", p=P),
            )

            def vt(jb):
                if jb < 4:
                    return v3a0[:, jb, :]
                if jb < 8:
                    return v3a1[:, jb - 4, :]
                return v3b[:, jb - 8, :]

            for jb in range(NJ):
                nc.vector.tensor_scalar_mul(
                    vt(jb), vt(jb).bitcast(F32), recipZ[:, jb : jb + 1]
                )

            for ic in range(NJ):
                po = ps.tile([P, 512], F32, tag="ps")
                for jb in range(ic + 1):
                    off = (
                        int(_ROW_OFF[jb])
                        + 512 * (ic // NI - jb // NI)
                        + P * (ic % NI)
                    )
                    nc.tensor.matmul(
                        po[:],
                        expST[:, off : off + P],
                        vt(jb),
                        start=(jb == 0),
                        stop=(jb == ic),
                    )
                st = stage.tile([P, 512], F32, tag="st")
                nc.vector.tensor_copy(st[:], po[:])
                nc.sync.dma_start(out_d[P * ic : P * (ic + 1), :], st[:])

    _split_excess_waits(nc)
    return nc


LAST_EXEC_NS = None
LAST_TRACE = None


def kernel(x, Wq, bq, Wk, bk, Wv, bv):
    global LAST_EXEC_NS, LAST_TRACE
    import os

    _apply_walrus_fixes()
    from concourse.bass_utils import run_bass_kernel_spmd

    x = np.asarray(x, dtype=np.float32)
    Wq = np.asarray(Wq, dtype=np.float32)
    Wk = np.asarray(Wk, dtype=np.float32)
    Wv = np.asarray(Wv, dtype=np.float32)
    bq = np.asarray(bq, dtype=np.float32)
    bk = np.asarray(bk, dtype=np.float32)
    bv = np.asarray(bv, dtype=np.float32)

    with_bias = bool(np.any(bq) or np.any(bk) or np.any(bv))

    # host-side layout prep (no FLOPs): transposes + mask/identity constants
    wqT = np.ascontiguousarray(Wq.T)
    wkT = np.ascontiguousarray(Wk.T)
    wvT = np.ascontiguousarray(Wv.T)
    r = np.arange(P)[:, None]
    c = np.arange(896)[None, :]
    maskE = np.where(c < 384 + r, np.float32(NEG), np.float32(0.0)).astype(np.float32)
    ident = np.eye(P, dtype=np.float32)

    in_maps = []
    for bidx in range(B):
        m = {
            "xT": np.ascontiguousarray(x[bidx].T),
            "wqT": wqT,
            "wkT": wkT,
            "wvT": wvT,
            "maskE": maskE,
            "ident": ident,
        }
        if with_bias:
            m["bq"] = bq.reshape(1, KS)
            m["bk"] = bk.reshape(1, KS)
            m["bv"] = bv.reshape(1, VS)
            m["ones"] = np.ones((1, 512), dtype=np.float32)
        in_maps.append(m)

    nc = _build(with_bias)
    kwargs = {}
    if os.environ.get("KERNEL_TRACE"):
        kwargs = {"trace": True, "tmpdir": os.environ.get("KERNEL_TRACE_DIR") or None}
    res = run_bass_kernel_spmd(nc, in_maps, core_ids=list(range(N_CORES)), **kwargs)
    LAST_EXEC_NS = res.exec_time_ns
    LAST_TRACE = res.instructions_and_trace[1] if res.instructions_and_trace else None
    attn = np.stack([res.results[bidx]["out"] for bidx in range(B)], axis=0)
    return np.concatenate([x, attn], axis=2)
